# revision 7
# baseline (speedup 1.0000x reference)
"""Low-rank layer y = x @ (U diag(s) V^T)^T on 8 TRN2 NeuronCores.

Data-parallel over batch (1 batch/core). Two thin matmuls per core:
  stage 1: t[r, n]  = sum_i (V*s)[i, r] * x[n, i]   (contraction i on partitions)
  stage 2: y[n, o]  = sum_r t[r, n] * U[o, r]       (contraction r on partitions)

Software-pipelined over 4 token groups of 512 so stage-2 of group g overlaps
stage-1 of group g+1: the tensor engine never waits on a phase boundary, input
DMA (SP ring) overlaps output DMA (ACT ring), and PSUM is split 2 banks
(stage-1 accumulators) + 6 banks (stage-2 output tiles). y is stored bf16 and
upcast on host, halving HBM write traffic. PSUM evacuation alternates between
vector and scalar engines.
"""

import numpy as np
import ml_dtypes

import concourse.bass as bass
import concourse.mybir as mybir
import concourse.tile as tile
from concourse.tile import ScopedClock
from concourse.bass_utils import run_bass_kernel_spmd

P = 128
B = 8
TOKENS = 2048
D_IN = 4096
D_OUT = 4096
R = 256
I_CHUNKS = D_IN // P  # 32
R_HALVES = R // P  # 2
G = 4  # token groups (pipeline stages)
NT = TOKENS // G  # 512 tokens per group
TB = NT // P  # 4 token blocks per group
O_TILE = 512
O_TILES = D_OUT // O_TILE  # 8
O_WAVE = 4  # out-tiles per PSUM wave in stage 2
X_SUBS = 8  # sub-DMAs per group's x tile (512 KiB each, contiguous in DRAM)
SUB_C = I_CHUNKS // X_SUBS  # i-chunks per sub-DMA


def _patched_drain_and_barrier(self, tick_clock, wait_clock):
    # This walrus build's CoreV3 CTRL lowering accepts at most one sync-wait
    # on the TileContext-exit SP Drain; split the global-clock waits across a
    # chain of SP nops (one wait each) emitted just before the drain.
    nc = self.nc
    lead = nc.sync.nop(nofuse=True, hint="tile_drain_wait_split")
    wait_clock.add_sem_waits(lead.ins, ScopedClock({None: tick_clock.global_clock}))
    si = lead.ins.sync_info
    waits = list(si.on_wait or [])
    if len(waits) > 1:
        si.on_wait = waits[:1]
        for w in waits[1:]:
            extra = nc.sync.nop(nofuse=True, hint="tile_drain_wait_split")
            esi = extra.ins.sync_info
            if esi is None:
                extra.ins.sync_info = mybir.SyncInfo(on_wait=[w], on_update=[])
            else:
                esi.on_wait = [w]
    nc.sync.drain()
    nc.all_engine_barrier()
    assert self.sems is not None
    popped = nc._tile_sem_poison_stack.pop()
    assert popped is self._sem_poison
    nc.clear_and_free_semaphores(list(self.sems.allocated().values()))
    nc.all_engine_barrier()


def _install_drain_patch():
    if not getattr(tile.TileContext, "_drain_patch_installed", False):
        tile.TileContext._drain_and_barrier = _patched_drain_and_barrier
        tile.TileContext._drain_patch_installed = True


def _legalize_waits(nc):
    # This walrus build accepts at most one sync-wait per instruction.
    # Hoist extra waits onto same-engine nops inserted just before the
    # offending instruction (same engine queue -> identical blocking).
    for fn in nc.m.functions:
        for bb in fn.blocks:
            new_list = []
            for inst in list(bb.instructions):
                si = inst.sync_info
                waits = list(si.on_wait) if si and si.on_wait else []
                if len(waits) > 1:
                    for w in waits[:-1]:
                        nop = nc.engines[inst.engine].nop(
                            nofuse=True, hint="wait_split"
                        )
                        cur = nc.cur_bb.bb.instructions
                        assert cur[-1] is nop.ins
                        cur.pop()
                        nsi = nop.ins.sync_info
                        if nsi is None:
                            nop.ins.sync_info = mybir.SyncInfo(
                                on_wait=[w], on_update=[]
                            )
                        else:
                            nsi.on_wait = [w]
                        new_list.append(nop.ins)
                    si.on_wait = [waits[-1]]
                new_list.append(inst)
            bb.instructions[:] = new_list


def _build(iodt=mybir.dt.bfloat16):
    f32 = mybir.dt.float32
    nc = bass.Bass()
    # xg rows: (group, sub, partition); cols: i-chunk-within-sub then token.
    # Each sub-DMA then reads a fully contiguous DRAM block (line-rate HBM).
    xg_d = nc.declare_dram_parameter(
        "xg", [G * X_SUBS * P, SUB_C * NT], iodt, isOutput=False
    )
    vs_d = nc.declare_dram_parameter("vs", [R_HALVES * P, I_CHUNKS * P], iodt, isOutput=False)
    ut_d = nc.declare_dram_parameter("ut", [P, R_HALVES, D_OUT], iodt, isOutput=False)
    y_d = nc.declare_dram_parameter("y", [TOKENS, D_OUT], iodt, isOutput=True)

    with tile.TileContext(nc) as tc:
        with (
            tc.tile_pool(name="consts", bufs=1) as consts,
            tc.tile_pool(name="xp", bufs=2) as xp,
            tc.tile_pool(name="tp", bufs=2) as tp,
            tc.tile_pool(name="yp", bufs=4) as yp,
            tc.tile_pool(name="pst", bufs=2, space="PSUM") as pst,
            tc.tile_pool(name="psy", bufs=6, space="PSUM") as psy,
        ):
            vs_sb = consts.tile([P, R_HALVES, I_CHUNKS * P], iodt)
            ut_sb = consts.tile([P, R_HALVES, D_OUT], iodt)
            # weights on the ACT ring; x loads own the SP ring
            for h in range(R_HALVES):
                nc.scalar.dma_start(
                    out=vs_sb[:, h, :], in_=vs_d[h * P : (h + 1) * P, :]
                )
            nc.scalar.dma_start(out=ut_sb[:], in_=ut_d[:])

            xg_sb = [None] * G
            t_sb = [None] * G

            def prefetch(g):
                xg_sb[g] = xp.tile([P, I_CHUNKS * NT], iodt, tag="xg", name="xg")
                w = SUB_C * NT
                for q in range(X_SUBS):
                    row = (g * X_SUBS + q) * P
                    nc.sync.dma_start(
                        out=xg_sb[g][:, q * w : (q + 1) * w],
                        in_=xg_d[row : row + P, :],
                    )

            def stage1(g):
                ps_t = [pst.tile([P, NT], f32, tag="pt", name="pt") for _ in range(R_HALVES)]
                xg = xg_sb[g]
                for h in range(R_HALVES):
                    for c in range(I_CHUNKS):
                        nc.tensor.matmul(
                            ps_t[h],
                            vs_sb[:, h, c * P : (c + 1) * P],
                            xg[:, c * NT : (c + 1) * NT],
                            start=(c == 0),
                            stop=(c == I_CHUNKS - 1),
                        )
                t_sb[g] = tp.tile([P, R_HALVES, NT], iodt, tag="t", name="t")
                for h in range(R_HALVES):
                    nc.vector.tensor_copy(out=t_sb[g][:, h, :], in_=ps_t[h])

            def stage2(g):
                for tb in range(TB):
                    y_sb = yp.tile([P, D_OUT], iodt, tag="yt", name="yt")
                    tsl = t_sb[g]
                    row = (g * TB + tb) * P
                    for w in range(O_TILES // O_WAVE):
                        ps_y = [None] * O_WAVE
                        for h in range(R_HALVES):
                            for j in range(O_WAVE):
                                ot = w * O_WAVE + j
                                if h == 0:
                                    # JIT alloc: each slot claimed right before
                                    # its first matmul so the wave can start as
                                    # prior-wave copies retire one by one
                                    ps_y[j] = psy.tile(
                                        [P, O_TILE], f32, tag="py", name="py"
                                    )
                                nc.tensor.matmul(
                                    ps_y[j],
                                    tsl[:, h, tb * P : (tb + 1) * P],
                                    ut_sb[:, h, ot * O_TILE : (ot + 1) * O_TILE],
                                    start=(h == 0),
                                    stop=(h == R_HALVES - 1),
                                )
                        for j in range(O_WAVE):
                            ot = w * O_WAVE + j
                            dst = y_sb[:, ot * O_TILE : (ot + 1) * O_TILE]
                            if ot % 2 == 0:
                                nc.vector.tensor_copy(out=dst, in_=ps_y[j])
                            else:
                                nc.scalar.copy(out=dst, in_=ps_y[j])
                        # store each half row-block as soon as its copies land
                        half = O_WAVE * O_TILE
                        nc.scalar.dma_start(
                            out=y_d[row : row + P, w * half : (w + 1) * half],
                            in_=y_sb[:, w * half : (w + 1) * half],
                        )

            # software pipeline: s1(0) s1(1) s2(0) s1(2) s2(1) s1(3) s2(2) s2(3)
            prefetch(0)
            stage1(0)
            prefetch(1)
            stage1(1)
            prefetch(2)
            stage2(0)
            stage1(2)
            prefetch(3)
            stage2(1)
            stage1(3)
            stage2(2)
            stage2(3)

    _legalize_waits(nc)
    return nc


_CACHED = {}


def kernel(x, u_approx, s_approx, v_approx, _trace=False):
    _install_drain_patch()
    bf16 = ml_dtypes.bfloat16

    vp = v_approx.astype(np.float32) * s_approx.astype(np.float32)[None, :]
    # vs[h*128+p, c*128+rr] = vp[c*128+p, h*128+rr]
    vs_host = np.ascontiguousarray(
        vp.reshape(I_CHUNKS, P, R_HALVES, P).transpose(2, 1, 0, 3).reshape(
            R_HALVES * P, I_CHUNKS * P
        )
    ).astype(bf16)
    # ut[p, h, o] = u[o, h*128+p]
    ut_host = np.ascontiguousarray(
        np.ascontiguousarray(u_approx.T).reshape(R_HALVES, P, D_OUT).transpose(1, 0, 2)
    ).astype(bf16)
    # xg[((g*X_SUBS+q)*128)+p, cc*512+n] = x[b, g*512+n, (q*SUB_C+cc)*128+p]
    xg = [
        np.ascontiguousarray(
            x[b]
            .reshape(G, NT, X_SUBS, SUB_C, P)
            .transpose(0, 2, 4, 3, 1)
            .reshape(G * X_SUBS * P, SUB_C * NT)
        ).astype(bf16)
        for b in range(B)
    ]
    in_maps = [{"xg": xg[b], "vs": vs_host, "ut": ut_host} for b in range(B)]

    if "nc" not in _CACHED:
        _CACHED["nc"] = _build()
    res = run_bass_kernel_spmd(_CACHED["nc"], in_maps, list(range(B)), trace=_trace)
    y = np.stack([res.results[b]["y"].astype(np.float32) for b in range(B)])
    if _trace:
        kernel.last_exec_time_ns = res.exec_time_ns
    return y


# revision 11
# speedup vs baseline: 1.0537x; 1.0537x over previous
"""Low-rank layer y = x @ (U diag(s) V^T)^T on 8 TRN2 NeuronCores.

Data-parallel over batch (1 batch/core). Two thin matmuls per core:
  stage 1: t[r, n]  = sum_i (V*s)[i, r] * x[n, i]   (contraction i on partitions)
  stage 2: y[n, o]  = sum_r t[r, n] * U[o, r]       (contraction r on partitions)

Software-pipelined over 4 token groups of 512 so stage-2 of group g overlaps
stage-1 of group g+1: the tensor engine never waits on a phase boundary, input
DMA (SP ring) overlaps output DMA (ACT ring), and PSUM is split 2 banks
(stage-1 accumulators) + 6 banks (stage-2 output tiles). y is stored bf16 and
upcast on host, halving HBM write traffic. PSUM evacuation alternates between
vector and scalar engines.
"""

import numpy as np
import ml_dtypes

import concourse.bass as bass
import concourse.mybir as mybir
import concourse.tile as tile
from concourse.tile import ScopedClock
from concourse.bass_utils import run_bass_kernel_spmd

P = 128
B = 8
TOKENS = 2048
D_IN = 4096
D_OUT = 4096
R = 256
I_CHUNKS = D_IN // P  # 32
R_HALVES = R // P  # 2
G = 4  # token groups (pipeline stages)
NT = TOKENS // G  # 512 tokens per group
TB = NT // P  # 4 token blocks per group
O_TILE = 512
O_TILES = D_OUT // O_TILE  # 8
O_WAVE = 4  # out-tiles per PSUM wave in stage 2
X_SUBS = 4  # sub-DMAs per group's x tile (1 MiB each, contiguous in DRAM)
SUB_C = I_CHUNKS // X_SUBS  # i-chunks per sub-DMA
WARM_MM = 16  # zero matmuls issued at t~7us to pre-warm the PE HAM clock gate


def _patched_drain_and_barrier(self, tick_clock, wait_clock):
    # This walrus build's CoreV3 CTRL lowering accepts at most one sync-wait
    # on the TileContext-exit SP Drain; split the global-clock waits across a
    # chain of SP nops (one wait each) emitted just before the drain.
    nc = self.nc
    lead = nc.sync.nop(nofuse=True, hint="tile_drain_wait_split")
    wait_clock.add_sem_waits(lead.ins, ScopedClock({None: tick_clock.global_clock}))
    si = lead.ins.sync_info
    waits = list(si.on_wait or [])
    if len(waits) > 1:
        si.on_wait = waits[:1]
        for w in waits[1:]:
            extra = nc.sync.nop(nofuse=True, hint="tile_drain_wait_split")
            esi = extra.ins.sync_info
            if esi is None:
                extra.ins.sync_info = mybir.SyncInfo(on_wait=[w], on_update=[])
            else:
                esi.on_wait = [w]
    nc.sync.drain()
    nc.all_engine_barrier()
    assert self.sems is not None
    popped = nc._tile_sem_poison_stack.pop()
    assert popped is self._sem_poison
    nc.clear_and_free_semaphores(list(self.sems.allocated().values()))
    nc.all_engine_barrier()


def _install_drain_patch():
    if not getattr(tile.TileContext, "_drain_patch_installed", False):
        tile.TileContext._drain_and_barrier = _patched_drain_and_barrier
        tile.TileContext._drain_patch_installed = True


def _legalize_waits(nc):
    # This walrus build accepts at most one sync-wait per instruction.
    # Hoist extra waits onto same-engine nops inserted just before the
    # offending instruction (same engine queue -> identical blocking).
    for fn in nc.m.functions:
        for bb in fn.blocks:
            new_list = []
            for inst in list(bb.instructions):
                si = inst.sync_info
                waits = list(si.on_wait) if si and si.on_wait else []
                if len(waits) > 1:
                    for w in waits[:-1]:
                        nop = nc.engines[inst.engine].nop(
                            nofuse=True, hint="wait_split"
                        )
                        cur = nc.cur_bb.bb.instructions
                        assert cur[-1] is nop.ins
                        cur.pop()
                        nsi = nop.ins.sync_info
                        if nsi is None:
                            nop.ins.sync_info = mybir.SyncInfo(
                                on_wait=[w], on_update=[]
                            )
                        else:
                            nsi.on_wait = [w]
                        new_list.append(nop.ins)
                    si.on_wait = [waits[-1]]
                new_list.append(inst)
            bb.instructions[:] = new_list


def _build(iodt=mybir.dt.bfloat16):
    f32 = mybir.dt.float32
    nc = bass.Bass()
    # xg rows: (group, sub, partition); cols: i-chunk-within-sub then token.
    # Each sub-DMA then reads a fully contiguous DRAM block (line-rate HBM).
    xg_d = nc.declare_dram_parameter(
        "xg", [G * X_SUBS * P, SUB_C * NT], iodt, isOutput=False
    )
    vs_d = nc.declare_dram_parameter("vs", [R_HALVES * P, I_CHUNKS * P], iodt, isOutput=False)
    ut_d = nc.declare_dram_parameter("ut", [P, R_HALVES, D_OUT], iodt, isOutput=False)
    y_d = nc.declare_dram_parameter("y", [TOKENS, D_OUT], iodt, isOutput=True)

    with tile.TileContext(nc) as tc:
        with (
            tc.tile_pool(name="consts", bufs=1) as consts,
            tc.tile_pool(name="xp", bufs=2) as xp,
            tc.tile_pool(name="tp", bufs=2) as tp,
            tc.tile_pool(name="yp", bufs=4) as yp,
            tc.tile_pool(name="pst", bufs=2, space="PSUM") as pst,
            tc.tile_pool(name="psy", bufs=6, space="PSUM") as psy,
        ):
            # PE warmup: ~3.4us of zero matmuls so the HAM clock-gate opens
            # to 2.4 GHz while the first weight/x DMAs are still in flight
            zt = consts.tile([P, P + O_TILE], iodt)
            nc.vector.memset(zt[:], 0.0)
            ps_warm = psy.tile([P, O_TILE], f32, tag="py", name="py")
            for _ in range(WARM_MM):
                nc.tensor.matmul(
                    ps_warm, zt[:, :P], zt[:, P:], start=True, stop=True
                )

            vs_sb = consts.tile([P, R_HALVES, I_CHUNKS * P], iodt)
            ut_sb = consts.tile([P, R_HALVES, D_OUT], iodt)
            # weights on the ACT ring; x loads own the SP ring
            for h in range(R_HALVES):
                nc.scalar.dma_start(
                    out=vs_sb[:, h, :], in_=vs_d[h * P : (h + 1) * P, :]
                )
            nc.scalar.dma_start(out=ut_sb[:], in_=ut_d[:])

            xg_sb = [None] * G
            t_sb = [None] * G

            def prefetch(g, eng):
                xg_sb[g] = xp.tile([P, I_CHUNKS * NT], iodt, tag="xg", name="xg")
                w = SUB_C * NT
                for q in range(X_SUBS):
                    row = (g * X_SUBS + q) * P
                    eng.dma_start(
                        out=xg_sb[g][:, q * w : (q + 1) * w],
                        in_=xg_d[row : row + P, :],
                    )

            def stage1(g):
                ps_t = [pst.tile([P, NT], f32, tag="pt", name="pt") for _ in range(R_HALVES)]
                xg = xg_sb[g]
                for h in range(R_HALVES):
                    for c in range(I_CHUNKS):
                        nc.tensor.matmul(
                            ps_t[h],
                            vs_sb[:, h, c * P : (c + 1) * P],
                            xg[:, c * NT : (c + 1) * NT],
                            start=(c == 0),
                            stop=(c == I_CHUNKS - 1),
                        )
                t_sb[g] = tp.tile([P, R_HALVES, NT], iodt, tag="t", name="t")
                for h in range(R_HALVES):
                    nc.vector.tensor_copy(out=t_sb[g][:, h, :], in_=ps_t[h])

            def stage2(g):
                for tb in range(TB):
                    y_sb = yp.tile([P, D_OUT], iodt, tag="yt", name="yt")
                    tsl = t_sb[g]
                    row = (g * TB + tb) * P
                    for w in range(O_TILES // O_WAVE):
                        ps_y = [None] * O_WAVE
                        for h in range(R_HALVES):
                            for j in range(O_WAVE):
                                ot = w * O_WAVE + j
                                if h == 0:
                                    # JIT alloc: each slot claimed right before
                                    # its first matmul so the wave can start as
                                    # prior-wave copies retire one by one
                                    ps_y[j] = psy.tile(
                                        [P, O_TILE], f32, tag="py", name="py"
                                    )
                                nc.tensor.matmul(
                                    ps_y[j],
                                    tsl[:, h, tb * P : (tb + 1) * P],
                                    ut_sb[:, h, ot * O_TILE : (ot + 1) * O_TILE],
                                    start=(h == 0),
                                    stop=(h == R_HALVES - 1),
                                )
                        for j in range(O_WAVE):
                            ot = w * O_WAVE + j
                            dst = y_sb[:, ot * O_TILE : (ot + 1) * O_TILE]
                            if ot % 2 == 0:
                                nc.vector.tensor_copy(out=dst, in_=ps_y[j])
                            else:
                                nc.scalar.copy(out=dst, in_=ps_y[j])
                    nc.scalar.dma_start(out=y_d[row : row + P, :], in_=y_sb[:])

            # software pipeline: s1(0) s1(1) s2(0) s1(2) s2(1) s1(3) s2(2) s2(3)
            # xg(1) rides the ACT ring (idle after weights until stores begin)
            # so both rings pull x during the ramp
            prefetch(0, nc.sync)
            prefetch(1, nc.scalar)
            stage1(0)
            stage1(1)
            prefetch(2, nc.sync)
            stage2(0)
            stage1(2)
            prefetch(3, nc.sync)
            stage2(1)
            stage1(3)
            stage2(2)
            stage2(3)

    _legalize_waits(nc)
    return nc


_CACHED = {}


def kernel(x, u_approx, s_approx, v_approx, _trace=False):
    _install_drain_patch()
    bf16 = ml_dtypes.bfloat16

    vp = v_approx.astype(np.float32) * s_approx.astype(np.float32)[None, :]
    # vs[h*128+p, c*128+rr] = vp[c*128+p, h*128+rr]
    vs_host = np.ascontiguousarray(
        vp.reshape(I_CHUNKS, P, R_HALVES, P).transpose(2, 1, 0, 3).reshape(
            R_HALVES * P, I_CHUNKS * P
        )
    ).astype(bf16)
    # ut[p, h, o] = u[o, h*128+p]
    ut_host = np.ascontiguousarray(
        np.ascontiguousarray(u_approx.T).reshape(R_HALVES, P, D_OUT).transpose(1, 0, 2)
    ).astype(bf16)
    # xg[((g*X_SUBS+q)*128)+p, cc*512+n] = x[b, g*512+n, (q*SUB_C+cc)*128+p]
    xg = [
        np.ascontiguousarray(
            x[b]
            .reshape(G, NT, X_SUBS, SUB_C, P)
            .transpose(0, 2, 4, 3, 1)
            .reshape(G * X_SUBS * P, SUB_C * NT)
        ).astype(bf16)
        for b in range(B)
    ]
    in_maps = [{"xg": xg[b], "vs": vs_host, "ut": ut_host} for b in range(B)]

    if "nc" not in _CACHED:
        _CACHED["nc"] = _build()
    res = run_bass_kernel_spmd(_CACHED["nc"], in_maps, list(range(B)), trace=_trace)
    y = np.stack([res.results[b]["y"].astype(np.float32) for b in range(B)])
    if _trace:
        kernel.last_exec_time_ns = res.exec_time_ns
    return y


# revision 18
# speedup vs baseline: 1.1401x; 1.0820x over previous
"""Low-rank layer y = x @ (U diag(s) V^T)^T on 8 TRN2 NeuronCores.

Data-parallel over batch (1 batch/core). Two thin matmuls per core:
  stage 1: t[r, n]  = sum_i (V*s)[i, r] * x[n, i]   (contraction i on partitions)
  stage 2: y[n, o]  = sum_r t[r, n] * U[o, r]       (contraction r on partitions)

Software-pipelined over 4 token groups of 512 so stage-2 of group g overlaps
stage-1 of group g+1: the tensor engine never waits on a phase boundary, input
DMA (SP ring) overlaps output DMA (ACT ring), and PSUM is split 2 banks
(stage-1 accumulators) + 6 banks (stage-2 output tiles). y is stored bf16 and
upcast on host, halving HBM write traffic. PSUM evacuation alternates between
vector and scalar engines.
"""

import numpy as np
import ml_dtypes

import concourse.bass as bass
import concourse.mybir as mybir
import concourse.tile as tile
from concourse.tile import ScopedClock
from concourse.bass_utils import run_bass_kernel_spmd

P = 128
B = 8
TOKENS = 2048
D_IN = 4096
D_OUT = 4096
R = 256
I_CHUNKS = D_IN // P  # 32
R_HALVES = R // P  # 2
G = 4  # token groups (pipeline stages)
NT = TOKENS // G  # 512 tokens per group
TB = NT // P  # 4 token blocks per group
O_TILE = 512
O_TILES = D_OUT // O_TILE  # 8
O_WAVE = 4  # out-tiles per PSUM wave in stage 2
X_SUBS = 4  # sub-DMAs per group's x tile (1 MiB each, contiguous in DRAM)
SUB_C = I_CHUNKS // X_SUBS  # i-chunks per sub-DMA
WARM_MM = 8  # zero matmuls issued at t~7us to pre-warm the PE HAM clock gate
FILL_MM = 4  # zero matmuls after each DMA-paced chunk octet (keep HAM open)


def _patched_drain_and_barrier(self, tick_clock, wait_clock):
    # This walrus build's CoreV3 CTRL lowering accepts at most one sync-wait
    # on the TileContext-exit SP Drain; split the global-clock waits across a
    # chain of SP nops (one wait each) emitted just before the drain.
    nc = self.nc
    lead = nc.sync.nop(nofuse=True, hint="tile_drain_wait_split")
    wait_clock.add_sem_waits(lead.ins, ScopedClock({None: tick_clock.global_clock}))
    si = lead.ins.sync_info
    waits = list(si.on_wait or [])
    if len(waits) > 1:
        si.on_wait = waits[:1]
        for w in waits[1:]:
            extra = nc.sync.nop(nofuse=True, hint="tile_drain_wait_split")
            esi = extra.ins.sync_info
            if esi is None:
                extra.ins.sync_info = mybir.SyncInfo(on_wait=[w], on_update=[])
            else:
                esi.on_wait = [w]
    nc.sync.drain()
    nc.all_engine_barrier()
    assert self.sems is not None
    popped = nc._tile_sem_poison_stack.pop()
    assert popped is self._sem_poison
    nc.clear_and_free_semaphores(list(self.sems.allocated().values()))
    nc.all_engine_barrier()


def _install_drain_patch():
    if not getattr(tile.TileContext, "_drain_patch_installed", False):
        tile.TileContext._drain_and_barrier = _patched_drain_and_barrier
        tile.TileContext._drain_patch_installed = True


def _legalize_waits(nc):
    # This walrus build accepts at most one sync-wait per instruction.
    # Hoist extra waits onto same-engine nops inserted just before the
    # offending instruction (same engine queue -> identical blocking).
    for fn in nc.m.functions:
        for bb in fn.blocks:
            new_list = []
            for inst in list(bb.instructions):
                si = inst.sync_info
                waits = list(si.on_wait) if si and si.on_wait else []
                if len(waits) > 1:
                    for w in waits[:-1]:
                        nop = nc.engines[inst.engine].nop(
                            nofuse=True, hint="wait_split"
                        )
                        cur = nc.cur_bb.bb.instructions
                        assert cur[-1] is nop.ins
                        cur.pop()
                        nsi = nop.ins.sync_info
                        if nsi is None:
                            nop.ins.sync_info = mybir.SyncInfo(
                                on_wait=[w], on_update=[]
                            )
                        else:
                            nsi.on_wait = [w]
                        new_list.append(nop.ins)
                    si.on_wait = [waits[-1]]
                new_list.append(inst)
            bb.instructions[:] = new_list


def _build(iodt=mybir.dt.bfloat16):
    f32 = mybir.dt.float32
    nc = bass.Bass()
    # xg rows: (group, sub, partition); cols: i-chunk-within-sub then token.
    # Each sub-DMA then reads a fully contiguous DRAM block (line-rate HBM).
    xg_d = nc.declare_dram_parameter(
        "xg", [G * X_SUBS * P, SUB_C * NT], iodt, isOutput=False
    )
    vs_d = nc.declare_dram_parameter("vs", [R_HALVES * P, I_CHUNKS * P], iodt, isOutput=False)
    ut_d = nc.declare_dram_parameter("ut", [P, R_HALVES, D_OUT], iodt, isOutput=False)
    y_d = nc.declare_dram_parameter("y", [TOKENS, D_OUT], iodt, isOutput=True)

    with tile.TileContext(nc) as tc:
        with (
            tc.tile_pool(name="consts", bufs=1) as consts,
            tc.tile_pool(name="xp", bufs=2) as xp,
            tc.tile_pool(name="tp", bufs=2) as tp,
            tc.tile_pool(name="yp", bufs=4) as yp,
            tc.tile_pool(name="pst", bufs=2, space="PSUM") as pst,
            tc.tile_pool(name="psy", bufs=6, space="PSUM") as psy,
        ):
            # PE warmup: ~3.4us of zero matmuls so the HAM clock-gate opens
            # to 2.4 GHz while the first weight/x DMAs are still in flight
            zt = consts.tile([P, P + O_TILE], iodt)
            nc.gpsimd.memset(zt[:], 0.0)
            ps_warm = psy.tile([P, O_TILE], f32, tag="py", name="py")

            def fill_mm(n):
                for _ in range(n):
                    nc.tensor.matmul(
                        ps_warm, zt[:, :P], zt[:, P:], start=True, stop=True
                    )

            fill_mm(WARM_MM)

            vs_sb = consts.tile([P, R_HALVES, I_CHUNKS * P], iodt)
            ut_sb = consts.tile([P, R_HALVES, D_OUT], iodt)
            # vs on the ACT ring ahead of xg(1); ut follows xg(1) (not needed
            # until stage2(0) at ~40us). x loads otherwise own the SP ring.
            for h in range(R_HALVES):
                nc.scalar.dma_start(
                    out=vs_sb[:, h, :], in_=vs_d[h * P : (h + 1) * P, :]
                )

            xg_sb = [None] * G
            t_sb = [None] * G

            def prefetch(g, eng):
                xg_sb[g] = xp.tile([P, I_CHUNKS * NT], iodt, tag="xg", name="xg")
                w = SUB_C * NT
                for q in range(X_SUBS):
                    row = (g * X_SUBS + q) * P
                    eng.dma_start(
                        out=xg_sb[g][:, q * w : (q + 1) * w],
                        in_=xg_d[row : row + P, :],
                    )

            def stage1(g, fillers=False):
                ps_t = [pst.tile([P, NT], f32, tag="pt", name="pt") for _ in range(R_HALVES)]
                xg = xg_sb[g]
                for h in range(R_HALVES):
                    for c in range(I_CHUNKS):
                        nc.tensor.matmul(
                            ps_t[h],
                            vs_sb[:, h, c * P : (c + 1) * P],
                            xg[:, c * NT : (c + 1) * NT],
                            start=(c == 0),
                            stop=(c == I_CHUNKS - 1),
                        )
                        # during the DMA-paced ramp, pad the x-sub-DMA wait at
                        # each octet boundary with zero matmuls so the PE never
                        # idles past the HAM MID window (would re-throttle)
                        if fillers and h == 0 and c % SUB_C == SUB_C - 1 and c != I_CHUNKS - 1:
                            fill_mm(FILL_MM)
                t_sb[g] = tp.tile([P, R_HALVES, NT], iodt, tag="t", name="t")
                for h in range(R_HALVES):
                    nc.vector.tensor_copy(out=t_sb[g][:, h, :], in_=ps_t[h])

            def stage2(g):
                # late-pipeline stores dispatch from the SP ring (loads done by
                # then) so the ACT engine isn't saturated by copies + dispatches
                st_eng = nc.scalar if g < 2 else nc.sync
                for tb in range(TB):
                    y_sb = yp.tile([P, D_OUT], iodt, tag="yt", name="yt")
                    tsl = t_sb[g]
                    row = (g * TB + tb) * P
                    for w in range(O_TILES // O_WAVE):
                        ps_y = [None] * O_WAVE
                        for h in range(R_HALVES):
                            for j in range(O_WAVE):
                                ot = w * O_WAVE + j
                                if h == 0:
                                    # JIT alloc: each slot claimed right before
                                    # its first matmul so the wave can start as
                                    # prior-wave copies retire one by one
                                    ps_y[j] = psy.tile(
                                        [P, O_TILE], f32, tag="py", name="py"
                                    )
                                nc.tensor.matmul(
                                    ps_y[j],
                                    tsl[:, h, tb * P : (tb + 1) * P],
                                    ut_sb[:, h, ot * O_TILE : (ot + 1) * O_TILE],
                                    start=(h == 0),
                                    stop=(h == R_HALVES - 1),
                                )
                        for j in range(O_WAVE):
                            ot = w * O_WAVE + j
                            dst = y_sb[:, ot * O_TILE : (ot + 1) * O_TILE]
                            if ot % 2 == 0:
                                nc.vector.tensor_copy(out=dst, in_=ps_y[j])
                            else:
                                nc.scalar.copy(out=dst, in_=ps_y[j])
                    st_eng.dma_start(out=y_d[row : row + P, :], in_=y_sb[:])

            # software pipeline: s1(0) s1(1) s2(0) s1(2) s2(1) s1(3) s2(2) s2(3)
            # xg(1) rides the ACT ring (idle after weights until stores begin)
            # so both rings pull x during the ramp
            prefetch(0, nc.sync)
            prefetch(1, nc.scalar)
            nc.scalar.dma_start(out=ut_sb[:], in_=ut_d[:])
            stage1(0, fillers=True)
            stage1(1, fillers=True)
            prefetch(2, nc.sync)
            stage2(0)
            stage1(2)
            prefetch(3, nc.sync)
            stage2(1)
            stage1(3)
            stage2(2)
            stage2(3)

    _legalize_waits(nc)
    return nc


_CACHED = {}


def kernel(x, u_approx, s_approx, v_approx, _trace=False):
    _install_drain_patch()
    bf16 = ml_dtypes.bfloat16

    vp = v_approx.astype(np.float32) * s_approx.astype(np.float32)[None, :]
    # vs[h*128+p, c*128+rr] = vp[c*128+p, h*128+rr]
    vs_host = np.ascontiguousarray(
        vp.reshape(I_CHUNKS, P, R_HALVES, P).transpose(2, 1, 0, 3).reshape(
            R_HALVES * P, I_CHUNKS * P
        )
    ).astype(bf16)
    # ut[p, h, o] = u[o, h*128+p]
    ut_host = np.ascontiguousarray(
        np.ascontiguousarray(u_approx.T).reshape(R_HALVES, P, D_OUT).transpose(1, 0, 2)
    ).astype(bf16)
    # xg[((g*X_SUBS+q)*128)+p, cc*512+n] = x[b, g*512+n, (q*SUB_C+cc)*128+p]
    xg = [
        np.ascontiguousarray(
            x[b]
            .reshape(G, NT, X_SUBS, SUB_C, P)
            .transpose(0, 2, 4, 3, 1)
            .reshape(G * X_SUBS * P, SUB_C * NT)
        ).astype(bf16)
        for b in range(B)
    ]
    in_maps = [{"xg": xg[b], "vs": vs_host, "ut": ut_host} for b in range(B)]

    if "nc" not in _CACHED:
        _CACHED["nc"] = _build()
    res = run_bass_kernel_spmd(_CACHED["nc"], in_maps, list(range(B)), trace=_trace)
    y = np.stack([res.results[b]["y"].astype(np.float32) for b in range(B)])
    if _trace:
        kernel.last_exec_time_ns = res.exec_time_ns
    return y
